# revision 7
# baseline (speedup 1.0000x reference)
"""Trainium2 Bass kernel for BinarySplitDecoder (binary-tree leaf probabilities).

Contract: kernel(x) takes the FULL input x [65536, 1023] fp32 and returns the
FULL output [65536, 1024] fp32 (leaf probabilities of a depth-10 binary split
tree, level-major node ordering).

Sharding: pure data parallel — batch dim split evenly across 8 NeuronCores.

Strategy (fp16 + block layout; memory-bound, ~33.5 MB of HBM I/O per core):
  - Host casts x to fp16 and permutes columns (within each tree level, a
    bit-reversal involution); the device returns fp16 leaves in bit-reversed
    ("block") order, which the host un-permutes + casts back to fp32. The
    2e-2 relative-error gate makes fp16 safe (measured ~1.5e-3).
  - Block layout: each tree step writes left children into a packed lower
    half and right children into a packed upper half (instead of interleaving
    with stride 2). Packed 2-byte operands let every tensor_tensor run in the
    DVE 2x_1p perf mode — 2x throughput; the interleaved store of the fp32
    baseline forced 1x mode.
  - right = cur - left replaces cur * (1 - a): no separate (1 - x) pass.
  - Rows processed in chunks of g*128; partition p / free-group i holds batch
    row off + p*g + i. Chunk loads split into three column pieces in separate
    tile pools (levels 0-7 / 8 / 9): the tree walk starts after ~25% of the
    chunk's bytes, and 3 bufs on the early pieces lets loads prefetch two
    chunks ahead. Early pieces issue from the ACT sequencer, the level-9
    piece from the Pool sequencer, stores from SP: descriptor generation
    and semaphore waits on one queue never block another's issue.
  - Output left/right halves are separate tiles: the left half's store (SP
    sequencer) overlaps the right half's subtract with no false WAR.
  - Small chunks at both ends shorten the pipeline ramp and the final store
    drain. DVE (2x) and DMA are both ~90% of the span; remaining cost is
    fixed framework preamble/teardown (~18 us).
"""

import numpy as np

import concourse.bacc as bacc
import concourse.bass as bass
import concourse.mybir as mybir
from concourse.tile import TileContext
from concourse.bass_utils import run_bass_kernel_spmd

TREE_DEPTH = 10
N_NODES = (1 << TREE_DEPTH) - 1  # 1023
N_LEAVES = 1 << TREE_DEPTH  # 1024
N_CORES = 8
P = 128  # SBUF partitions
H = N_LEAVES // 2  # 512


def _bitrev(n: int, bits: int) -> int:
    r = 0
    for _ in range(bits):
        r = (r << 1) | (n & 1)
        n >>= 1
    return r


def _col_perm() -> np.ndarray:
    """xp[:, base+p] = x[:, base+rev_s(p)]: per-level bit-reversal so the
    block-layout walk consumes alphas from contiguous slices."""
    perm = np.arange(N_NODES)
    for s in range(TREE_DEPTH):
        base = (1 << s) - 1
        for p in range(1 << s):
            perm[base + p] = base + _bitrev(p, s)
    return perm


COL_PERM = _col_perm()
# block position j holds standard leaf rev(j); rev is an involution
OUT_PERM = np.array([_bitrev(m, TREE_DEPTH) for m in range(N_LEAVES)])


def build_nc(rows_per_core: int, G: int = 16) -> bass.Bass:
    """Per-core Bass program: DRAM "x" [rows_per_core, 1023] fp16 (columns
    pre-permuted) -> DRAM "y" [rows_per_core, 1024] fp16 (block leaf order).
    """
    units = rows_per_core // P
    # small chunks at both ends: short pipeline ramp AND short store drain
    chunks = [1, 2, 4, 8] + [G] * ((units - 16) // G) + [1]
    assert sum(chunks) == units, (rows_per_core, chunks)
    f16 = mybir.dt.float16

    nc = bacc.Bacc("TRN2", target_bir_lowering=False, debug=False)
    x = nc.declare_dram_parameter("x", [rows_per_core, N_NODES], f16, isOutput=False)
    y = nc.declare_dram_parameter("y", [rows_per_core, N_LEAVES], f16, isOutput=True)

    def x_view(off, g, c0, c1):
        return x[off : off + g * P, c0:c1].rearrange("(p g) n -> p g n", g=g, p=P)

    def y_view(off, g, c0, c1):
        return y[off : off + g * P, c0:c1].rearrange("(p g) m -> p g m", g=g, p=P)

    with TileContext(nc) as tc:
        with (
            tc.tile_pool(name="xa", bufs=3) as xap,  # levels 0-7 alphas
            tc.tile_pool(name="xb", bufs=3) as xbp,  # level 8 alphas
            tc.tile_pool(name="xc", bufs=2) as xcp,  # level 9 alphas
            tc.tile_pool(name="outL", bufs=2) as outlp,
            tc.tile_pool(name="outR", bufs=2) as outrp,
            # bufs=2: with one buffer, chunk c+1's level-0 write must wait
            # for the level-9 reads of chunk c (WAR) — a per-chunk stall.
            tc.tile_pool(name="cur", bufs=2) as curp,
        ):
            off = 0
            for g in chunks:
                xa = xap.tile([P, g, 255], f16, tag="xa")
                xb = xbp.tile([P, g, 256], f16, tag="xb")
                xc = xcp.tile([P, g, 512], f16, tag="xc")
                nc.scalar.dma_start(out=xa[:], in_=x_view(off, g, 0, 255))
                nc.scalar.dma_start(out=xb[:], in_=x_view(off, g, 255, 511))
                nc.gpsimd.dma_start(out=xc[:], in_=x_view(off, g, 511, 1023))

                out_l = outlp.tile([P, g, H], f16, tag="yl")
                out_r = outrp.tile([P, g, H], f16, tag="yr")
                cur = None
                for d in range(TREE_DEPTH):
                    L = 1 << d
                    if d == TREE_DEPTH - 1:
                        a = xc[:, :, 0:L]
                        left = out_l[:, :, 0:L]
                        right = out_r[:, :, 0:L]
                    else:
                        # ping-pong intermediate levels between two shared
                        # slots (sized by the largest level using each tag)
                        nxt = curp.tile([P, g, 2 * L], f16, tag=f"cur{d % 2}")
                        a = (
                            xb[:, :, 0:L]
                            if d == TREE_DEPTH - 2
                            else xa[:, :, L - 1 : 2 * L - 1]
                        )
                        left = nxt[:, :, 0:L]
                        right = nxt[:, :, L : 2 * L]
                    if d == 0:
                        nc.vector.tensor_copy(out=left, in_=a)
                        nc.vector.tensor_scalar(
                            out=right,
                            in0=a,
                            scalar1=-1.0,
                            scalar2=1.0,
                            op0=mybir.AluOpType.mult,
                            op1=mybir.AluOpType.add,
                        )
                    else:
                        nc.vector.tensor_mul(out=left, in0=cur, in1=a)
                        if d == TREE_DEPTH - 1:
                            # the left half of the leaves is final: start
                            # draining it while the right half is computed
                            nc.sync.dma_start(
                                out=y_view(off, g, 0, H), in_=out_l[:]
                            )
                        nc.vector.tensor_tensor(
                            out=right, in0=cur, in1=left, op=mybir.AluOpType.subtract
                        )
                    if d < TREE_DEPTH - 1:
                        cur = nxt

                nc.sync.dma_start(out=y_view(off, g, H, N_LEAVES), in_=out_r[:])
                off += g * P

    nc.compile()
    return nc


def _run(x: np.ndarray, **spmd_kwargs):
    """Shard x, run the Bass kernel on all 8 cores, return (y, BassKernelResults)."""
    x = np.asarray(x, dtype=np.float32)
    B = x.shape[0]
    assert B % N_CORES == 0 and x.shape[1] == N_NODES
    rows_per_core = B // N_CORES

    xh = np.ascontiguousarray(x[:, COL_PERM].astype(np.float16))

    nc = build_nc(rows_per_core)
    core_ids = list(range(N_CORES))
    in_maps = [
        {"x": xh[i * rows_per_core : (i + 1) * rows_per_core]} for i in core_ids
    ]
    res = run_bass_kernel_spmd(nc, in_maps, core_ids, **spmd_kwargs)
    out = np.concatenate([r["y"] for r in res.results], axis=0)
    out = out[:, OUT_PERM].astype(np.float32)
    return out, res


def kernel(x: np.ndarray) -> np.ndarray:
    return _run(x)[0]


# revision 8
# speedup vs baseline: 1.0087x; 1.0087x over previous
"""Trainium2 Bass kernel for BinarySplitDecoder (binary-tree leaf probabilities).

Contract: kernel(x) takes the FULL input x [65536, 1023] fp32 and returns the
FULL output [65536, 1024] fp32 (leaf probabilities of a depth-10 binary split
tree, level-major node ordering).

Sharding: pure data parallel — batch dim split evenly across 8 NeuronCores.

Strategy (fp16 + block layout; memory-bound, ~33.5 MB of HBM I/O per core):
  - Host casts x to fp16 and permutes columns (within each tree level, a
    bit-reversal involution); the device returns fp16 leaves in bit-reversed
    ("block") order, which the host un-permutes + casts back to fp32. The
    2e-2 relative-error gate makes fp16 safe (measured ~1.5e-3).
  - Block layout: each tree step writes left children into a packed lower
    half and right children into a packed upper half (instead of interleaving
    with stride 2). Packed 2-byte operands let every tensor_tensor run in the
    DVE 2x_1p perf mode — 2x throughput; the interleaved store of the fp32
    baseline forced 1x mode.
  - right = cur - left replaces cur * (1 - a): no separate (1 - x) pass.
  - Rows processed in chunks of g*128; partition p / free-group i holds batch
    row off + p*g + i. The input is staged on the host as three separate
    DRAM arrays (levels 0-7 / level 8 / level 9 alphas) and the output as
    two (left / right leaf halves): every DMA is a fully merged 2D access
    pattern (one descriptor per partition — a column-sliced 3D pattern costs
    ~6x more sequencer descriptor-gen time and stalls the pipeline), the
    tree walk starts after ~25% of a chunk's bytes, the left half's store
    overlaps the right half's subtract, and 3 bufs on the early pieces lets
    loads prefetch two chunks ahead.
  - Early pieces issue from the ACT sequencer, the level-9 piece from the
    Pool sequencer, stores from SP: one queue's semaphore waits never block
    another's issue.
  - Small chunks at both ends shorten the pipeline ramp and the final store
    drain. DVE (2x) and DMA are both ~90% of the span; remaining cost is
    fixed framework preamble/teardown (~18 us).
"""

import numpy as np

import concourse.bacc as bacc
import concourse.bass as bass
import concourse.mybir as mybir
from concourse.tile import TileContext
from concourse.bass_utils import run_bass_kernel_spmd

TREE_DEPTH = 10
N_NODES = (1 << TREE_DEPTH) - 1  # 1023
N_LEAVES = 1 << TREE_DEPTH  # 1024
N_CORES = 8
P = 128  # SBUF partitions
H = N_LEAVES // 2  # 512
NA, NB, NC = 255, 256, 512  # levels 0-7, level 8, level 9 alpha counts


def _bitrev(n: int, bits: int) -> int:
    r = 0
    for _ in range(bits):
        r = (r << 1) | (n & 1)
        n >>= 1
    return r


def _col_perm() -> np.ndarray:
    """xp[:, base+p] = x[:, base+rev_s(p)]: per-level bit-reversal so the
    block-layout walk consumes alphas from contiguous slices."""
    perm = np.arange(N_NODES)
    for s in range(TREE_DEPTH):
        base = (1 << s) - 1
        for p in range(1 << s):
            perm[base + p] = base + _bitrev(p, s)
    return perm


COL_PERM = _col_perm()
# block position j holds standard leaf rev(j); rev is an involution
OUT_PERM = np.array([_bitrev(m, TREE_DEPTH) for m in range(N_LEAVES)])


def build_nc(rows_per_core: int, G: int = 16) -> bass.Bass:
    """Per-core Bass program. DRAM inputs "xa"/"xb"/"xc" hold the fp16
    column-permuted alphas of levels 0-7 / 8 / 9; outputs "yl"/"yr" are the
    left/right halves of the block-ordered fp16 leaves."""
    units = rows_per_core // P
    # small chunks at both ends: short pipeline ramp AND short store drain
    chunks = [1, 2, 4, 8] + [G] * ((units - 16) // G) + [1]
    assert sum(chunks) == units, (rows_per_core, chunks)
    f16 = mybir.dt.float16

    nc = bacc.Bacc("TRN2", target_bir_lowering=False, debug=False)
    xa_d = nc.declare_dram_parameter("xa", [rows_per_core, NA], f16, isOutput=False)
    xb_d = nc.declare_dram_parameter("xb", [rows_per_core, NB], f16, isOutput=False)
    xc_d = nc.declare_dram_parameter("xc", [rows_per_core, NC], f16, isOutput=False)
    yl_d = nc.declare_dram_parameter("yl", [rows_per_core, H], f16, isOutput=True)
    yr_d = nc.declare_dram_parameter("yr", [rows_per_core, H], f16, isOutput=True)

    def view(t, off, g):
        return t[off : off + g * P, :].rearrange("(p g) n -> p (g n)", g=g, p=P)

    with TileContext(nc) as tc:
        with (
            tc.tile_pool(name="xa", bufs=3) as xap,
            tc.tile_pool(name="xb", bufs=3) as xbp,
            tc.tile_pool(name="xc", bufs=2) as xcp,
            tc.tile_pool(name="outL", bufs=2) as outlp,
            tc.tile_pool(name="outR", bufs=2) as outrp,
            # bufs=2: with one buffer, chunk c+1's level-0 write must wait
            # for the level-9 reads of chunk c (WAR) — a per-chunk stall.
            tc.tile_pool(name="cur", bufs=2) as curp,
        ):
            off = 0
            for g in chunks:
                xa = xap.tile([P, g, NA], f16, tag="xa")
                xb = xbp.tile([P, g, NB], f16, tag="xb")
                xc = xcp.tile([P, g, NC], f16, tag="xc")
                nc.scalar.dma_start(out=xa[:], in_=view(xa_d, off, g))
                nc.scalar.dma_start(out=xb[:], in_=view(xb_d, off, g))
                nc.gpsimd.dma_start(out=xc[:], in_=view(xc_d, off, g))

                out_l = outlp.tile([P, g, H], f16, tag="yl")
                out_r = outrp.tile([P, g, H], f16, tag="yr")
                cur = None
                for d in range(TREE_DEPTH):
                    L = 1 << d
                    if d == TREE_DEPTH - 1:
                        a = xc[:, :, 0:L]
                        left = out_l[:, :, 0:L]
                        right = out_r[:, :, 0:L]
                    else:
                        # ping-pong intermediate levels between two shared
                        # slots (sized by the largest level using each tag)
                        nxt = curp.tile([P, g, 2 * L], f16, tag=f"cur{d % 2}")
                        a = (
                            xb[:, :, 0:L]
                            if d == TREE_DEPTH - 2
                            else xa[:, :, L - 1 : 2 * L - 1]
                        )
                        left = nxt[:, :, 0:L]
                        right = nxt[:, :, L : 2 * L]
                    if d == 0:
                        nc.vector.tensor_copy(out=left, in_=a)
                        nc.vector.tensor_scalar(
                            out=right,
                            in0=a,
                            scalar1=-1.0,
                            scalar2=1.0,
                            op0=mybir.AluOpType.mult,
                            op1=mybir.AluOpType.add,
                        )
                    else:
                        nc.vector.tensor_mul(out=left, in0=cur, in1=a)
                        if d == TREE_DEPTH - 1:
                            # the left half of the leaves is final: start
                            # draining it while the right half is computed
                            nc.sync.dma_start(
                                out=view(yl_d, off, g), in_=out_l[:]
                            )
                        nc.vector.tensor_tensor(
                            out=right, in0=cur, in1=left, op=mybir.AluOpType.subtract
                        )
                    if d < TREE_DEPTH - 1:
                        cur = nxt

                nc.sync.dma_start(out=view(yr_d, off, g), in_=out_r[:])
                off += g * P

    nc.compile()
    return nc


def _run(x: np.ndarray, **spmd_kwargs):
    """Shard x, run the Bass kernel on all 8 cores, return (y, BassKernelResults)."""
    x = np.asarray(x, dtype=np.float32)
    B = x.shape[0]
    assert B % N_CORES == 0 and x.shape[1] == N_NODES
    rows_per_core = B // N_CORES

    xh = x[:, COL_PERM].astype(np.float16)
    xa_h = np.ascontiguousarray(xh[:, :NA])
    xb_h = np.ascontiguousarray(xh[:, NA : NA + NB])
    xc_h = np.ascontiguousarray(xh[:, NA + NB :])

    nc = build_nc(rows_per_core)
    core_ids = list(range(N_CORES))
    in_maps = [
        {
            "xa": xa_h[i * rows_per_core : (i + 1) * rows_per_core],
            "xb": xb_h[i * rows_per_core : (i + 1) * rows_per_core],
            "xc": xc_h[i * rows_per_core : (i + 1) * rows_per_core],
        }
        for i in core_ids
    ]
    res = run_bass_kernel_spmd(nc, in_maps, core_ids, **spmd_kwargs)
    out = np.concatenate(
        [np.hstack([r["yl"], r["yr"]]) for r in res.results], axis=0
    )
    out = out[:, OUT_PERM].astype(np.float32)
    return out, res


def kernel(x: np.ndarray) -> np.ndarray:
    return _run(x)[0]
